# revision 28
# baseline (speedup 1.0000x reference)
"""HardTripletLoss Trainium2 kernel.

Reference computation (B=256, C=1000, D=300):
  relations[b,c] = ||emb[b*C+c] - att[b*C+c] + 1e-6||_2          [B, C]
  hardest_positive[c] = max_b relations[b,c] * onehot(labels)[b,c]
  mx[c]              = max_b relations[b,c]
  hardest_negative[c] = min_b (relations[b,c] + mx[c]*onehot[b,c])
  loss = sum(relu(hp - hn + 1)) / (count(relu(...) > 1e-16) + 1e-16)

Sharding: data-parallel over B across 8 cores (32 b's per core, each a
contiguous 32000-row chunk of the (B*C, D) tensors). Each core computes
squared distances and 4 per-class partial reductions [C]:
  cmax = max_b rel_sq            pmax = max_b over positives of rel_sq
  umin = min_b over negatives    mmin = min_b over positives
(masking is exact: +/-1e30 select-style masks via min/max ALU ops, no
additive-cancellation error). Host all-reduces the [4, C] partials over
cores, takes sqrt (monotone, commutes with max/min), and finishes the
tiny [C]-sized tail: hn = min(umin, cmax_r + mmin, loss scalar.

DMA strategy (memory-bound: 76.8 MB/core of f32 input). Measured HW
facts driving the design:
  - the binding resource is the DMA->SBUF write path, ~208 GB/s/core
    aggregate; the HBM read side and the 16-engine array can do ~410
    GB/s. So the loads CAST f32->f16 in flight (SWDGE CCE): the write
    side halves and reads run at the ~410 GB/s engine roofline.
    (f16 rounding is ~1e-4 relative on the distances; gate is 2e-2.)
  - per-queue throughput is the next limit: one SWDGE ring sustains
    ~78-100 GB/s (4 fixed DMA engines each, ~31-descriptor serial
    blocks per engine). All 64 chunk loads are spread over the 4 SWDGE
    rings -- plain gpsimd dma_start is pinned to ring 0 by bass, so the
    InstDMACopy queue name is retargeted post-hoc to qPoolDynamic{1,2,3}.
    The 2 HWDGE queues share 5 engines with the rings and cannot cast;
    they carry only the tiny mask/out transfers.
  - partition p holds R=8 consecutive rows (c = 8p + r): each per-b load
    is a dense 1.2 MB 2D DMA, 9.6 KB contiguous HBM lines, 125 descs.
  - the last 2 chunks are split into partition-quarter sub-DMAs, one
    per ring, so the final drain uses all 16 engines instead of one
    ring's 4; io bufs=7 gives the lookahead that keeps rings fed (6 and
    8 are both measurably worse).
Compute: DVE subtracts e-a (f16); ACT Square+accum_out makes the row
sums for 6 of 8 rows (PSUM junk output keeps its writes off SBUF), DVE
mult+reduce the other 2, so neither engine trails the DMA stream. The
masked per-class reductions run in two waves (b<16 hidden mid-stream).
"""

import numpy as np

B, C, D = 256, 1000, 300
M = 8            # cores
BL = B // M      # 32 local anchors per core
P = 100          # partitions; partition p holds classes c = R*p + r
R = C // P       # 10 consecutive rows per partition
BIG = 1.0e30
EPS_PD = 1e-6
MARGIN = 1.0

_STATE = {}


def _build():
    import concourse.tile as tile
    from concourse import bacc, mybir

    nc = bacc.Bacc("TRN2", target_bir_lowering=False, debug=False,
                   num_devices=M, num_swdge_queues=4)
    dt = mybir.dt.float32
    emb = nc.dram_tensor("emb", [BL * C, D], dt, kind="ExternalInput").ap()
    att = nc.dram_tensor("att", [BL * C, D], dt, kind="ExternalInput").ap()
    msk = nc.dram_tensor("msk", [P, R * BL], dt, kind="ExternalInput").ap()
    out = nc.dram_tensor("out", [P, 4 * R], dt, kind="ExternalOutput").ap()

    emb_v = emb.rearrange("(b p r) d -> b p r d", b=BL, p=P, r=R)
    att_v = att.rearrange("(b p r) d -> b p r d", b=BL, p=P, r=R)
    f16 = mybir.dt.float16

    Alu = mybir.AluOpType
    Act = mybir.ActivationFunctionType
    Ax = mybir.AxisListType

    def swdge(dst, src, q):
        """gpsimd dma_start retargeted to SWDGE ring q (0-3)."""
        inst = nc.gpsimd.dma_start(dst, src)
        if q:
            inst.ins.queue = f"qPoolDynamic{q}"
        return inst

    with tile.TileContext(nc) as tc:
        with (
            tc.tile_pool(name="io", bufs=7) as io_pool,
            tc.tile_pool(name="dif", bufs=2) as dif_pool,
            tc.tile_pool(name="small", bufs=1) as small_pool,
            tc.psum_pool(name="ps", bufs=2) as ps_pool,
        ):
            mask_t = small_pool.tile([P, R * BL], dt, tag="mask")
            nc.sync.dma_start(mask_t[:], msk[:])
            mask2_t = small_pool.tile([P, R * BL], dt, tag="mask2")
            nc.vector.tensor_scalar_mul(mask2_t[:], mask_t[:], -1.0)
            # rel_t column b*R + r holds rel_sq of (b, c=R*p+r)
            rel_t = small_pool.tile([P, BL * R], dt, tag="rel")
            # ACT's mandatory main output goes to PSUM: junk f32 writes to
            # SBUF (1.2 MB/chunk) were stealing SBUF write bandwidth from DMA
            junk_t = ps_pool.tile([P, D], dt, tag="junk", bufs=1)
            sq_t = ps_pool.tile([P, 4, D], dt, tag="sq", bufs=1)
            waste_t = small_pool.tile([P, BL // 2], dt, tag="waste")
            partA_t = small_pool.tile([P, 4 * R], dt, tag="partA")
            partB_t = small_pool.tile([P, 4 * R], dt, tag="partB")

            def reduce_wave(part, b0, b1):
                """Masked per-class partial min/max over b in [b0, b1).

                Wave 1 runs mid-stream (hidden under the DMA); only wave 2
                trails the last chunk.
                """
                n = b1 - b0
                for r in range(R):
                    rel_r = rel_t[:, b0 * R + r: b1 * R: R]
                    m_r = mask_t[:, r * BL + b0: r * BL + b1]
                    m2_r = mask2_t[:, r * BL + b0: r * BL + b1]
                    w = waste_t[:, :n]
                    nc.vector.tensor_reduce(
                        part[:, 0 * R + r: 0 * R + r + 1], rel_r,
                        axis=Ax.X, op=Alu.max)
                    # masking via min/max with +-1e30 select masks is exact
                    nc.vector.tensor_tensor(w, rel_r, m_r, op=Alu.min)
                    nc.vector.tensor_reduce(
                        part[:, 1 * R + r: 1 * R + r + 1], w,
                        axis=Ax.X, op=Alu.max)
                    nc.vector.tensor_tensor(w, rel_r, m_r, op=Alu.max)
                    nc.vector.tensor_reduce(
                        part[:, 2 * R + r: 2 * R + r + 1], w,
                        axis=Ax.X, op=Alu.min)
                    nc.vector.tensor_tensor(w, rel_r, m2_r, op=Alu.max)
                    nc.vector.tensor_reduce(
                        part[:, 3 * R + r: 3 * R + r + 1], w,
                        axis=Ax.X, op=Alu.min)

            # f32 -> f16 cast during the DMA (SWDGE CCE): halves the
            # SBUF-write bytes; the f16 rounding is ~1e-4 relative on the
            # distances, far under the 2e-2 gate.
            # Each SWDGE ring feeds 4 fixed DMA engines in ~31-descriptor
            # serial blocks, so a ring's last full-size DMA drains ~12 us on
            # one engine. The final 2 chunks are split into partition-quarter
            # sub-DMAs, one per ring, so the tail drains on all 16 engines.
            psl = [slice(0, 32), slice(32, 64), slice(64, 96), slice(96, P)]
            ph = [slice(0, 63), slice(63, P)]
            for b in range(BL):
                # each load is split across ring pairs so all 4 rings carry
                # equal backlogs and their queues run dry together at the end
                # (whole-chunk-per-ring skewed the final drain onto one ring)
                e_t = io_pool.tile([P, R, D], f16, tag="e")
                a_t = io_pool.tile([P, R, D], f16, tag="a")
                if b < BL - 2:
                    for j in range(2):
                        swdge(e_t[ph[j]], emb_v[b][ph[j]], (2 * b + j) % 4)
                    for j in range(2):
                        swdge(a_t[ph[j]], att_v[b][ph[j]], (2 * b + 2 + j) % 4)
                else:
                    for k in range(4):
                        swdge(e_t[psl[k]], emb_v[b][psl[k]], k)
                    for k in range(4):
                        swdge(a_t[psl[k]], att_v[b][psl[k]], (k + 2) % 4)
                d_t = dif_pool.tile([P, R, D], f16, tag="d")
                nc.vector.tensor_sub(d_t[:], e_t[:], a_t[:])
                # split the square+rowsum rows between ACT and DVE; even the
                # split (4/4) for the last 2 chunks so both engines finish
                # together right after the final load
                n_dve = 2 if b < BL - 2 else 4
                for r in range(R - n_dve):
                    # accum_out gives the 300-wide row-sum for free (f32)
                    nc.scalar.activation(
                        junk_t[:], d_t[:, r, :], Act.Square,
                        bias=0.0, scale=1.0,
                        accum_out=rel_t[:, b * R + r: b * R + r + 1],
                    )
                nc.vector.tensor_tensor(
                    sq_t[:, :n_dve], d_t[:, R - n_dve:, :],
                    d_t[:, R - n_dve:, :], op=Alu.mult)
                nc.vector.tensor_reduce(
                    rel_t[:, b * R + R - n_dve: b * R + R], sq_t[:, :n_dve],
                    axis=Ax.X, op=Alu.add)
                if b == BL // 2 - 1:
                    reduce_wave(partA_t, 0, BL // 2)

            reduce_wave(partB_t, BL // 2, BL)
            # combine the two waves: cols [0:2R] are maxes, [2R:4R] are mins
            nc.vector.tensor_tensor(
                partA_t[:, : 2 * R], partA_t[:, : 2 * R],
                partB_t[:, : 2 * R], op=Alu.max)
            nc.vector.tensor_tensor(
                partA_t[:, 2 * R:], partA_t[:, 2 * R:],
                partB_t[:, 2 * R:], op=Alu.min)
            nc.sync.dma_start(out[:], partA_t[:])
    nc.compile()
    return nc


def _get_nc():
    if "nc" not in _STATE:
        _STATE["nc"] = _build()
    return _STATE["nc"]


def _make_masks(labels_np):
    """Per-core select masks msk[p, r*BL+b] = +BIG if labels[b]==R*p+r else -BIG."""
    masks = []
    c_of_pr = R * np.arange(P)[:, None] + np.arange(R)[None, :]     # [P, R]
    for m in range(M):
        lb = labels_np[m * BL:(m + 1) * BL].astype(np.int64)        # [BL]
        match = c_of_pr[:, :, None] == lb[None, None, :]            # [P, R, BL]
        mask = np.where(match, np.float32(BIG), np.float32(-BIG))
        masks.append(np.ascontiguousarray(mask.reshape(P, R * BL),
                                          dtype=np.float32))
    return masks


def _partials_from_out(o):
    """Device out [P, 4R] (col k*R+r, class c = R*p + r) -> [4, C] float64."""
    return np.transpose(o.astype(np.float64).reshape(P, 4, R),
                        (1, 0, 2)).reshape(4, C)


def _run_device(attributes, embeddings, labels_np, trace=False):
    from concourse.bass_utils import run_bass_kernel_spmd
    nc = _get_nc()
    masks = _make_masks(labels_np)
    in_maps = []
    for m in range(M):
        sl = slice(m * BL * C, (m + 1) * BL * C)
        in_maps.append({
            "emb": embeddings[sl],
            "att": attributes[sl],
            "msk": masks[m],
        })
    return run_bass_kernel_spmd(nc, in_maps, list(range(M)), trace=trace)


def _combine(results):
    """All-reduce the per-core [P, 4R] partials and finish the loss on host."""
    cmax = np.full(C, -np.inf)
    pmax = np.full(C, -np.inf)
    umin = np.full(C, np.inf)
    mmin = np.full(C, np.inf)
    for m in range(M):
        pk = _partials_from_out(results[m]["out"])
        cmax = np.maximum(cmax, pk[0])
        pmax = np.maximum(pmax, pk[1])
        umin = np.minimum(umin, pk[2])
        mmin = np.minimum(mmin, pk[3])
    # squared space -> distances (max/min commute with sqrt on [0, inf))
    mx = np.sqrt(np.maximum(cmax, 0.0))
    hp = np.sqrt(np.maximum(pmax, 0.0))       # -BIG (no positive) -> 0
    umin_r = np.sqrt(np.maximum(umin, 0.0))   # +BIG sentinel stays huge
    mmin_r = np.sqrt(np.maximum(mmin, 0.0))
    hn = np.minimum(umin_r, mx + mmin_r)
    triplet = np.maximum(hp - hn + MARGIN, 0.0)
    num_hard = np.sum(triplet > 1e-16)
    loss = np.sum(triplet) / (num_hard + 1e-16)
    return np.float32(loss)


def kernel(attributes, embeddings, labels):
    attributes = np.ascontiguousarray(np.asarray(attributes, dtype=np.float32))
    embeddings = np.ascontiguousarray(np.asarray(embeddings, dtype=np.float32))
    labels_np = np.asarray(labels)
    res = _run_device(attributes, embeddings, labels_np)
    return _combine(res.results)


# revision 29
# speedup vs baseline: 1.3256x; 1.3256x over previous
"""HardTripletLoss Trainium2 kernel.

Reference computation (B=256, C=1000, D=300):
  relations[b,c] = ||emb[b*C+c] - att[b*C+c] + 1e-6||_2          [B, C]
  hardest_positive[c] = max_b relations[b,c] * onehot(labels)[b,c]
  mx[c]              = max_b relations[b,c]
  hardest_negative[c] = min_b (relations[b,c] + mx[c]*onehot[b,c])
  loss = sum(relu(hp - hn + 1)) / (count(relu(...) > 1e-16) + 1e-16)

Sharding: data-parallel over B across 8 cores (32 b's per core, each a
contiguous 32000-row chunk of the (B*C, D) tensors). Each core computes
squared distances and 4 per-class partial reductions [C]:
  cmax = max_b rel_sq            pmax = max_b over positives of rel_sq
  umin = min_b over negatives    mmin = min_b over positives
(masking is exact: +/-1e30 select-style masks via min/max ALU ops, no
additive-cancellation error). Host all-reduces the [4, C] partials over
cores, takes sqrt (monotone, commutes with max/min), and finishes the
tiny [C]-sized tail: hn = min(umin, cmax_r + mmin, loss scalar.

DMA strategy (memory-bound: 76.8 MB/core of f32 input). Measured HW
facts driving the design:
  - the binding resource is the DMA->SBUF write path, ~208 GB/s/core
    aggregate; the HBM read side and the 16-engine array can do ~410
    GB/s. So the loads CAST f32->f16 in flight (SWDGE CCE): the write
    side halves and reads run at the ~410 GB/s engine roofline.
    (f16 rounding is ~1e-4 relative on the distances; gate is 2e-2.)
  - per-queue throughput is the next limit: one SWDGE ring sustains
    ~78-100 GB/s (4 fixed DMA engines each, ~31-descriptor serial
    blocks per engine). All 64 chunk loads are spread over the 4 SWDGE
    rings -- plain gpsimd dma_start is pinned to ring 0 by bass, so the
    InstDMACopy queue name is retargeted post-hoc to qPoolDynamic{1,2,3}.
    The 2 HWDGE queues share 5 engines with the rings and cannot cast;
    they carry only the tiny mask/out transfers.
  - partition p holds R=8 consecutive rows (c = 8p + r): each per-b load
    is a dense 1.2 MB 2D DMA, 9.6 KB contiguous HBM lines, 125 descs.
  - the last 2 chunks are split into partition-quarter sub-DMAs, one
    per ring, so the final drain uses all 16 engines instead of one
    ring's 4; io bufs=7 gives the lookahead that keeps rings fed (6 and
    8 are both measurably worse).
Compute: DVE subtracts e-a (f16); ACT Square+accum_out makes the row
sums for 6 of 8 rows (PSUM junk output keeps its writes off SBUF), DVE
mult+reduce the other 2, so neither engine trails the DMA stream. The
masked per-class reductions run in two waves (b<16 hidden mid-stream).
"""

import numpy as np

B, C, D = 256, 1000, 300
M = 8            # cores
BL = B // M      # 32 local anchors per core
P = 100          # partitions; partition p holds classes c = R*p + r
R = C // P       # 10 consecutive rows per partition
BIG = 1.0e30
EPS_PD = 1e-6
MARGIN = 1.0

_STATE = {}


def _build():
    import concourse.tile as tile
    from concourse import bacc, mybir

    nc = bacc.Bacc("TRN2", target_bir_lowering=False, debug=False,
                   num_devices=M, num_swdge_queues=4)
    dt = mybir.dt.float32
    emb = nc.dram_tensor("emb", [BL * C, D], dt, kind="ExternalInput").ap()
    att = nc.dram_tensor("att", [BL * C, D], dt, kind="ExternalInput").ap()
    msk = nc.dram_tensor("msk", [P, R * BL], dt, kind="ExternalInput").ap()
    out = nc.dram_tensor("out", [P, 4 * R], dt, kind="ExternalOutput").ap()

    emb_v = emb.rearrange("(b p r) d -> b p r d", b=BL, p=P, r=R)
    att_v = att.rearrange("(b p r) d -> b p r d", b=BL, p=P, r=R)
    f16 = mybir.dt.float16

    Alu = mybir.AluOpType
    Act = mybir.ActivationFunctionType
    Ax = mybir.AxisListType

    def swdge(dst, src, q):
        """gpsimd dma_start retargeted to SWDGE ring q (0-3)."""
        inst = nc.gpsimd.dma_start(dst, src)
        if q:
            inst.ins.queue = f"qPoolDynamic{q}"
        return inst

    with tile.TileContext(nc) as tc:
        with (
            tc.tile_pool(name="io", bufs=7) as io_pool,
            tc.tile_pool(name="dif", bufs=2) as dif_pool,
            tc.tile_pool(name="small", bufs=1) as small_pool,
            tc.psum_pool(name="ps", bufs=2) as ps_pool,
        ):
            mask_t = small_pool.tile([P, R * BL], dt, tag="mask")
            nc.sync.dma_start(mask_t[:], msk[:])
            mask2_t = small_pool.tile([P, R * BL], dt, tag="mask2")
            nc.vector.tensor_scalar_mul(mask2_t[:], mask_t[:], -1.0)
            # rel_t column b*R + r holds rel_sq of (b, c=R*p+r)
            rel_t = small_pool.tile([P, BL * R], dt, tag="rel")
            # ACT's mandatory main output goes to PSUM: junk f32 writes to
            # SBUF (1.2 MB/chunk) were stealing SBUF write bandwidth from DMA
            junk_t = ps_pool.tile([P, D], dt, tag="junk", bufs=1)
            sq_t = ps_pool.tile([P, 4, D], dt, tag="sq", bufs=1)
            waste_t = small_pool.tile([P, BL // 2], dt, tag="waste")
            partA_t = small_pool.tile([P, 4 * R], dt, tag="partA")
            partB_t = small_pool.tile([P, 4 * R], dt, tag="partB")

            def reduce_wave(part, b0, b1):
                """Masked per-class partial min/max over b in [b0, b1).

                Wave 1 runs mid-stream (hidden under the DMA); only wave 2
                trails the last chunk.
                """
                n = b1 - b0
                for r in range(R):
                    rel_r = rel_t[:, b0 * R + r: b1 * R: R]
                    m_r = mask_t[:, r * BL + b0: r * BL + b1]
                    m2_r = mask2_t[:, r * BL + b0: r * BL + b1]
                    w = waste_t[:, :n]
                    nc.vector.tensor_reduce(
                        part[:, 0 * R + r: 0 * R + r + 1], rel_r,
                        axis=Ax.X, op=Alu.max)
                    # masking via min/max with +-1e30 select masks is exact
                    nc.vector.tensor_tensor(w, rel_r, m_r, op=Alu.min)
                    nc.vector.tensor_reduce(
                        part[:, 1 * R + r: 1 * R + r + 1], w,
                        axis=Ax.X, op=Alu.max)
                    nc.vector.tensor_tensor(w, rel_r, m_r, op=Alu.max)
                    nc.vector.tensor_reduce(
                        part[:, 2 * R + r: 2 * R + r + 1], w,
                        axis=Ax.X, op=Alu.min)
                    nc.vector.tensor_tensor(w, rel_r, m2_r, op=Alu.max)
                    nc.vector.tensor_reduce(
                        part[:, 3 * R + r: 3 * R + r + 1], w,
                        axis=Ax.X, op=Alu.min)

            # f32 -> f16 cast during the DMA (SWDGE CCE): halves the
            # SBUF-write bytes; the f16 rounding is ~1e-4 relative on the
            # distances, far under the 2e-2 gate.
            # Each SWDGE ring feeds 4 fixed DMA engines in ~31-descriptor
            # serial blocks, so a ring's last full-size DMA drains ~12 us on
            # one engine. The final 2 chunks are split into partition-quarter
            # sub-DMAs, one per ring, so the tail drains on all 16 engines.
            psl = [slice(0, 32), slice(32, 64), slice(64, 96), slice(96, P)]
            for b in range(BL):
                # whole contiguous 1.2 MB b-chunk, 9.6 KB partition lines
                e_t = io_pool.tile([P, R, D], f16, tag="e")
                a_t = io_pool.tile([P, R, D], f16, tag="a")
                if b < BL - 2:
                    swdge(e_t[:], emb_v[b], (2 * b) % 4)
                    swdge(a_t[:], att_v[b], (2 * b + 1) % 4)
                else:
                    for k in range(4):
                        swdge(e_t[psl[k]], emb_v[b][psl[k]], k)
                    for k in range(4):
                        swdge(a_t[psl[k]], att_v[b][psl[k]], (k + 2) % 4)
                d_t = dif_pool.tile([P, R, D], f16, tag="d")
                nc.vector.tensor_sub(d_t[:], e_t[:], a_t[:])
                # split the square+rowsum rows between ACT and DVE; even the
                # split (4/4) for the last 2 chunks so both engines finish
                # together right after the final load
                n_dve = 2 if b < BL - 6 else 4
                for r in range(R - n_dve):
                    # accum_out gives the 300-wide row-sum for free (f32)
                    nc.scalar.activation(
                        junk_t[:], d_t[:, r, :], Act.Square,
                        bias=0.0, scale=1.0,
                        accum_out=rel_t[:, b * R + r: b * R + r + 1],
                    )
                nc.vector.tensor_tensor(
                    sq_t[:, :n_dve], d_t[:, R - n_dve:, :],
                    d_t[:, R - n_dve:, :], op=Alu.mult)
                nc.vector.tensor_reduce(
                    rel_t[:, b * R + R - n_dve: b * R + R], sq_t[:, :n_dve],
                    axis=Ax.X, op=Alu.add)
                if b == BL // 2 - 1:
                    reduce_wave(partA_t, 0, BL // 2)

            reduce_wave(partB_t, BL // 2, BL)
            # combine the two waves: cols [0:2R] are maxes, [2R:4R] are mins
            nc.vector.tensor_tensor(
                partA_t[:, : 2 * R], partA_t[:, : 2 * R],
                partB_t[:, : 2 * R], op=Alu.max)
            nc.vector.tensor_tensor(
                partA_t[:, 2 * R:], partA_t[:, 2 * R:],
                partB_t[:, 2 * R:], op=Alu.min)
            nc.sync.dma_start(out[:], partA_t[:])
    nc.compile()
    return nc


def _get_nc():
    if "nc" not in _STATE:
        _STATE["nc"] = _build()
    return _STATE["nc"]


def _make_masks(labels_np):
    """Per-core select masks msk[p, r*BL+b] = +BIG if labels[b]==R*p+r else -BIG."""
    masks = []
    c_of_pr = R * np.arange(P)[:, None] + np.arange(R)[None, :]     # [P, R]
    for m in range(M):
        lb = labels_np[m * BL:(m + 1) * BL].astype(np.int64)        # [BL]
        match = c_of_pr[:, :, None] == lb[None, None, :]            # [P, R, BL]
        mask = np.where(match, np.float32(BIG), np.float32(-BIG))
        masks.append(np.ascontiguousarray(mask.reshape(P, R * BL),
                                          dtype=np.float32))
    return masks


def _partials_from_out(o):
    """Device out [P, 4R] (col k*R+r, class c = R*p + r) -> [4, C] float64."""
    return np.transpose(o.astype(np.float64).reshape(P, 4, R),
                        (1, 0, 2)).reshape(4, C)


def _run_device(attributes, embeddings, labels_np, trace=False):
    from concourse.bass_utils import run_bass_kernel_spmd
    nc = _get_nc()
    masks = _make_masks(labels_np)
    in_maps = []
    for m in range(M):
        sl = slice(m * BL * C, (m + 1) * BL * C)
        in_maps.append({
            "emb": embeddings[sl],
            "att": attributes[sl],
            "msk": masks[m],
        })
    return run_bass_kernel_spmd(nc, in_maps, list(range(M)), trace=trace)


def _combine(results):
    """All-reduce the per-core [P, 4R] partials and finish the loss on host."""
    cmax = np.full(C, -np.inf)
    pmax = np.full(C, -np.inf)
    umin = np.full(C, np.inf)
    mmin = np.full(C, np.inf)
    for m in range(M):
        pk = _partials_from_out(results[m]["out"])
        cmax = np.maximum(cmax, pk[0])
        pmax = np.maximum(pmax, pk[1])
        umin = np.minimum(umin, pk[2])
        mmin = np.minimum(mmin, pk[3])
    # squared space -> distances (max/min commute with sqrt on [0, inf))
    mx = np.sqrt(np.maximum(cmax, 0.0))
    hp = np.sqrt(np.maximum(pmax, 0.0))       # -BIG (no positive) -> 0
    umin_r = np.sqrt(np.maximum(umin, 0.0))   # +BIG sentinel stays huge
    mmin_r = np.sqrt(np.maximum(mmin, 0.0))
    hn = np.minimum(umin_r, mx + mmin_r)
    triplet = np.maximum(hp - hn + MARGIN, 0.0)
    num_hard = np.sum(triplet > 1e-16)
    loss = np.sum(triplet) / (num_hard + 1e-16)
    return np.float32(loss)


def kernel(attributes, embeddings, labels):
    attributes = np.ascontiguousarray(np.asarray(attributes, dtype=np.float32))
    embeddings = np.ascontiguousarray(np.asarray(embeddings, dtype=np.float32))
    labels_np = np.asarray(labels)
    res = _run_device(attributes, embeddings, labels_np)
    return _combine(res.results)


# revision 31
# speedup vs baseline: 1.3649x; 1.0296x over previous
"""HardTripletLoss Trainium2 kernel.

Reference computation (B=256, C=1000, D=300):
  relations[b,c] = ||emb[b*C+c] - att[b*C+c] + 1e-6||_2          [B, C]
  hardest_positive[c] = max_b relations[b,c] * onehot(labels)[b,c]
  mx[c]              = max_b relations[b,c]
  hardest_negative[c] = min_b (relations[b,c] + mx[c]*onehot[b,c])
  loss = sum(relu(hp - hn + 1)) / (count(relu(...) > 1e-16) + 1e-16)

Sharding: data-parallel over B across 8 cores (32 b's per core, each a
contiguous 32000-row chunk of the (B*C, D) tensors). Each core computes
squared distances and 4 per-class partial reductions [C]:
  cmax = max_b rel_sq            pmax = max_b over positives of rel_sq
  umin = min_b over negatives    mmin = min_b over positives
(masking is exact: +/-1e30 select-style masks via min/max ALU ops, no
additive-cancellation error). Host all-reduces the [4, C] partials over
cores, takes sqrt (monotone, commutes with max/min), and finishes the
tiny [C]-sized tail: hn = min(umin, cmax_r + mmin, loss scalar.

DMA strategy (memory-bound: 76.8 MB/core of f32 input). Measured HW
facts driving the design:
  - the binding resource is the DMA->SBUF write path, ~208 GB/s/core
    aggregate; the HBM read side and the 16-engine array can do ~410
    GB/s. So the loads CAST f32->f16 in flight (SWDGE CCE): the write
    side halves and reads run at the ~410 GB/s engine roofline.
    (f16 rounding is ~1e-4 relative on the distances; gate is 2e-2.)
  - per-queue throughput is the next limit: one SWDGE ring sustains
    ~78-100 GB/s (4 fixed DMA engines each, ~31-descriptor serial
    blocks per engine). All 64 chunk loads are spread over the 4 SWDGE
    rings -- plain gpsimd dma_start is pinned to ring 0 by bass, so the
    InstDMACopy queue name is retargeted post-hoc to qPoolDynamic{1,2,3}.
    The 2 HWDGE queues share 5 engines with the rings and cannot cast;
    they carry only the tiny mask/out transfers.
  - partition p holds R=8 consecutive rows (c = 8p + r): each per-b load
    is a dense 1.2 MB 2D DMA, 9.6 KB contiguous HBM lines, 125 descs.
  - the last 2 chunks are split into partition-quarter sub-DMAs, one
    per ring, so the final drain uses all 16 engines instead of one
    ring's 4; io bufs=7 gives the lookahead that keeps rings fed (6 and
    8 are both measurably worse).
Compute: DVE subtracts e-a (f16); ACT Square+accum_out makes the row
sums for 6 of 8 rows (PSUM junk output keeps its writes off SBUF), DVE
mult+reduce the other 2, so neither engine trails the DMA stream. The
masked per-class reductions run in two waves (b<16 hidden mid-stream).
"""

import numpy as np

B, C, D = 256, 1000, 300
M = 8            # cores
BL = B // M      # 32 local anchors per core
P = 100          # partitions; partition p holds classes c = R*p + r
R = C // P       # 10 consecutive rows per partition
BIG = 1.0e30
EPS_PD = 1e-6
MARGIN = 1.0

_STATE = {}


def _build():
    import concourse.tile as tile
    from concourse import bacc, mybir

    nc = bacc.Bacc("TRN2", target_bir_lowering=False, debug=False,
                   num_devices=M, num_swdge_queues=4)
    dt = mybir.dt.float32
    emb = nc.dram_tensor("emb", [BL * C, D], dt, kind="ExternalInput").ap()
    att = nc.dram_tensor("att", [BL * C, D], dt, kind="ExternalInput").ap()
    msk = nc.dram_tensor("msk", [P, R * BL], dt, kind="ExternalInput").ap()
    out = nc.dram_tensor("out", [P, 4 * R], dt, kind="ExternalOutput").ap()

    emb_v = emb.rearrange("(b p r) d -> b p r d", b=BL, p=P, r=R)
    att_v = att.rearrange("(b p r) d -> b p r d", b=BL, p=P, r=R)
    f16 = mybir.dt.float16

    Alu = mybir.AluOpType
    Act = mybir.ActivationFunctionType
    Ax = mybir.AxisListType

    def swdge(dst, src, q):
        """gpsimd dma_start retargeted to SWDGE ring q (0-3)."""
        inst = nc.gpsimd.dma_start(dst, src)
        if q:
            inst.ins.queue = f"qPoolDynamic{q}"
        return inst

    with tile.TileContext(nc) as tc:
        with (
            tc.tile_pool(name="io", bufs=7) as io_pool,
            tc.tile_pool(name="dif", bufs=2) as dif_pool,
            tc.tile_pool(name="small", bufs=1) as small_pool,
            tc.psum_pool(name="ps", bufs=2) as ps_pool,
        ):
            mask_t = small_pool.tile([P, R * BL], dt, tag="mask")
            nc.sync.dma_start(mask_t[:], msk[:])
            mask2_t = small_pool.tile([P, R * BL], dt, tag="mask2")
            nc.vector.tensor_scalar_mul(mask2_t[:], mask_t[:], -1.0)
            # rel_t column b*R + r holds rel_sq of (b, c=R*p+r)
            rel_t = small_pool.tile([P, BL * R], dt, tag="rel")
            # ACT's mandatory main output goes to PSUM: junk f32 writes to
            # SBUF (1.2 MB/chunk) were stealing SBUF write bandwidth from DMA
            junk_t = ps_pool.tile([P, D], dt, tag="junk", bufs=1)
            sq_t = ps_pool.tile([P, 4, D], dt, tag="sq", bufs=1)
            waste_t = small_pool.tile([P, BL // 2], dt, tag="waste")
            partA_t = small_pool.tile([P, 4 * R], dt, tag="partA")
            partB_t = small_pool.tile([P, 4 * R], dt, tag="partB")

            def reduce_wave(part, b0, b1):
                """Masked per-class partial min/max over b in [b0, b1).

                Wave 1 runs mid-stream (hidden under the DMA); only wave 2
                trails the last chunk.
                """
                n = b1 - b0
                for r in range(R):
                    rel_r = rel_t[:, b0 * R + r: b1 * R: R]
                    m_r = mask_t[:, r * BL + b0: r * BL + b1]
                    m2_r = mask2_t[:, r * BL + b0: r * BL + b1]
                    w = waste_t[:, :n]
                    nc.vector.tensor_reduce(
                        part[:, 0 * R + r: 0 * R + r + 1], rel_r,
                        axis=Ax.X, op=Alu.max)
                    # masking via min/max with +-1e30 select masks is exact
                    nc.vector.tensor_tensor(w, rel_r, m_r, op=Alu.min)
                    nc.vector.tensor_reduce(
                        part[:, 1 * R + r: 1 * R + r + 1], w,
                        axis=Ax.X, op=Alu.max)
                    nc.vector.tensor_tensor(w, rel_r, m_r, op=Alu.max)
                    nc.vector.tensor_reduce(
                        part[:, 2 * R + r: 2 * R + r + 1], w,
                        axis=Ax.X, op=Alu.min)
                    nc.vector.tensor_tensor(w, rel_r, m2_r, op=Alu.max)
                    nc.vector.tensor_reduce(
                        part[:, 3 * R + r: 3 * R + r + 1], w,
                        axis=Ax.X, op=Alu.min)

            # f32 -> f16 cast during the DMA (SWDGE CCE): halves the
            # SBUF-write bytes; the f16 rounding is ~1e-4 relative on the
            # distances, far under the 2e-2 gate.
            # Each SWDGE ring feeds 4 fixed DMA engines in ~31-descriptor
            # serial blocks, so a ring's last full-size DMA drains ~12 us on
            # one engine. The final 2 chunks are split into partition-quarter
            # sub-DMAs, one per ring, so the tail drains on all 16 engines.
            for b in range(BL):
                # whole contiguous 1.2 MB b-chunk, 9.6 KB partition lines.
                # The last chunk pair gets dedicated buffers: those loads
                # issue with no tile-free wait (the in-order issue engine
                # stalled ~8us there otherwise), and the 4 full loads land
                # one per ring so the final drain runs ring-parallel.
                if b < BL - 2:
                    e_t = io_pool.tile([P, R, D], f16, tag="e")
                    a_t = io_pool.tile([P, R, D], f16, tag="a")
                else:
                    e_t = io_pool.tile([P, R, D], f16, tag=f"e_tl{b & 1}",
                                       bufs=1)
                    a_t = io_pool.tile([P, R, D], f16, tag=f"a_tl{b & 1}",
                                       bufs=1)
                swdge(e_t[:], emb_v[b], (2 * b) % 4)
                swdge(a_t[:], att_v[b], (2 * b + 1) % 4)
                d_t = dif_pool.tile([P, R, D], f16, tag="d")
                nc.vector.tensor_sub(d_t[:], e_t[:], a_t[:])
                # split the square+rowsum rows between ACT and DVE; even the
                # split (4/4) for the last 6 chunks so tail buffers free
                # sooner and the final loads issue without stalling
                n_dve = 2 if b < BL - 6 else 4
                for r in range(R - n_dve):
                    # accum_out gives the 300-wide row-sum for free (f32)
                    nc.scalar.activation(
                        junk_t[:], d_t[:, r, :], Act.Square,
                        bias=0.0, scale=1.0,
                        accum_out=rel_t[:, b * R + r: b * R + r + 1],
                    )
                nc.vector.tensor_tensor(
                    sq_t[:, :n_dve], d_t[:, R - n_dve:, :],
                    d_t[:, R - n_dve:, :], op=Alu.mult)
                nc.vector.tensor_reduce(
                    rel_t[:, b * R + R - n_dve: b * R + R], sq_t[:, :n_dve],
                    axis=Ax.X, op=Alu.add)
                if b == BL // 2 - 1:
                    reduce_wave(partA_t, 0, BL // 2)

            reduce_wave(partB_t, BL // 2, BL)
            # combine the two waves: cols [0:2R] are maxes, [2R:4R] are mins
            nc.vector.tensor_tensor(
                partA_t[:, : 2 * R], partA_t[:, : 2 * R],
                partB_t[:, : 2 * R], op=Alu.max)
            nc.vector.tensor_tensor(
                partA_t[:, 2 * R:], partA_t[:, 2 * R:],
                partB_t[:, 2 * R:], op=Alu.min)
            nc.sync.dma_start(out[:], partA_t[:])
    nc.compile()
    return nc


def _get_nc():
    if "nc" not in _STATE:
        _STATE["nc"] = _build()
    return _STATE["nc"]


def _make_masks(labels_np):
    """Per-core select masks msk[p, r*BL+b] = +BIG if labels[b]==R*p+r else -BIG."""
    masks = []
    c_of_pr = R * np.arange(P)[:, None] + np.arange(R)[None, :]     # [P, R]
    for m in range(M):
        lb = labels_np[m * BL:(m + 1) * BL].astype(np.int64)        # [BL]
        match = c_of_pr[:, :, None] == lb[None, None, :]            # [P, R, BL]
        mask = np.where(match, np.float32(BIG), np.float32(-BIG))
        masks.append(np.ascontiguousarray(mask.reshape(P, R * BL),
                                          dtype=np.float32))
    return masks


def _partials_from_out(o):
    """Device out [P, 4R] (col k*R+r, class c = R*p + r) -> [4, C] float64."""
    return np.transpose(o.astype(np.float64).reshape(P, 4, R),
                        (1, 0, 2)).reshape(4, C)


def _run_device(attributes, embeddings, labels_np, trace=False):
    from concourse.bass_utils import run_bass_kernel_spmd
    nc = _get_nc()
    masks = _make_masks(labels_np)
    in_maps = []
    for m in range(M):
        sl = slice(m * BL * C, (m + 1) * BL * C)
        in_maps.append({
            "emb": embeddings[sl],
            "att": attributes[sl],
            "msk": masks[m],
        })
    return run_bass_kernel_spmd(nc, in_maps, list(range(M)), trace=trace)


def _combine(results):
    """All-reduce the per-core [P, 4R] partials and finish the loss on host."""
    cmax = np.full(C, -np.inf)
    pmax = np.full(C, -np.inf)
    umin = np.full(C, np.inf)
    mmin = np.full(C, np.inf)
    for m in range(M):
        pk = _partials_from_out(results[m]["out"])
        cmax = np.maximum(cmax, pk[0])
        pmax = np.maximum(pmax, pk[1])
        umin = np.minimum(umin, pk[2])
        mmin = np.minimum(mmin, pk[3])
    # squared space -> distances (max/min commute with sqrt on [0, inf))
    mx = np.sqrt(np.maximum(cmax, 0.0))
    hp = np.sqrt(np.maximum(pmax, 0.0))       # -BIG (no positive) -> 0
    umin_r = np.sqrt(np.maximum(umin, 0.0))   # +BIG sentinel stays huge
    mmin_r = np.sqrt(np.maximum(mmin, 0.0))
    hn = np.minimum(umin_r, mx + mmin_r)
    triplet = np.maximum(hp - hn + MARGIN, 0.0)
    num_hard = np.sum(triplet > 1e-16)
    loss = np.sum(triplet) / (num_hard + 1e-16)
    return np.float32(loss)


def kernel(attributes, embeddings, labels):
    attributes = np.ascontiguousarray(np.asarray(attributes, dtype=np.float32))
    embeddings = np.ascontiguousarray(np.asarray(embeddings, dtype=np.float32))
    labels_np = np.asarray(labels)
    res = _run_device(attributes, embeddings, labels_np)
    return _combine(res.results)
